# revision 6
# baseline (speedup 1.0000x reference)
"""Trainium2 Bass kernel for nn_CrossAttentionModule (cross-attention token
re-weighting): per batch, L2-normalize 196 tokens of class/query feats over
C=640 channels, corr = ct @ qt^T, tiny MLP on corr means -> kernel vector,
softmax(corr @ kernel / T) -> per-token attention, out = feat * (1 + attn).

Sharding: pure data parallel, B=512 -> 64 batches on each of 8 NeuronCores.
"""
import numpy as np

try:
    import concourse.bass as bass
except ImportError:  # fresh grading dir: toolchain lives in /opt/trn_rl_repo
    import sys
    sys.path.insert(0, "/opt/trn_rl_repo")
    import concourse.bass as bass

import bass_rust
import concourse.mybir as mybir
from concourse import tile
from concourse.bass_utils import run_bass_kernel_spmd
from concourse.vector_clock import ScopedClock

F32 = mybir.dt.float32
AF = mybir.ActivationFunctionType
ALU = mybir.AluOpType

C = 640          # channels
T = 196          # tokens (14*14)
NCH = 5          # C / 128 chunks
TA, TB = 128, 68  # token chunks
INV_TEMP = 40.0  # 1 / 0.025
N_CORES = 8


def _patched_drain_and_barrier(self, tick_clock, wait_clock):
    # Walrus here rejects >2 sync waits on one instruction ("Too many sync
    # wait commands"). Emit one wait_ge per semaphore, then a bare drain.
    probe = self.nc.sync.nop()
    wait_clock.add_sem_waits(probe.ins, ScopedClock({None: tick_clock.global_clock}))
    si = probe.ins.sync_info
    waits = list(si.on_wait) if si is not None else []
    probe.ins.sync_info = bass_rust.SyncInfo(
        on_wait=[], on_update=list(si.on_update) if si is not None else []
    )
    handles = {h.name: h for h in self.sems.allocated().values()}
    for w in waits:
        self.nc.sync.wait_ge(handles[w.ant_name], w.wait_value)
    self.nc.sync.drain()
    self.nc.all_engine_barrier()
    popped = self.nc._tile_sem_poison_stack.pop()
    assert popped is self._sem_poison
    self.nc.clear_and_free_semaphores(list(self.sems.allocated().values()))
    self.nc.all_engine_barrier()


tile.TileContext._drain_and_barrier = _patched_drain_and_barrier

WAIT_LIMIT = 1  # max sem waits walrus accepts on one instruction


def _split_waits(nc, limit=WAIT_LIMIT):
    """Walrus rejects instructions carrying more than `limit` sync waits.
    Move excess waits onto same-engine NoOps inserted just before."""
    n_split = 0
    for fn in nc.m.functions:
        for blk in fn.blocks:
            il = blk.instructions  # live list
            idx = 0
            while idx < len(il):
                inst = il[idx]
                si = getattr(inst, "sync_info", None)
                if si is not None and len(si.on_wait) > limit:
                    waits = list(si.on_wait)
                    inst.sync_info = bass_rust.SyncInfo(
                        on_wait=waits[:limit], on_update=list(si.on_update))
                    extra = waits[limit:]
                    pos = idx
                    for j in range(0, len(extra), limit):
                        nop = mybir.InstNoOp(
                            name=f"wsplit-{nc.next_id()}", ins=[], outs=[])
                        nop.engine = inst.engine
                        nop.sync_info = bass_rust.SyncInfo(
                            on_wait=extra[j:j + limit], on_update=[])
                        il.insert(pos, nop)
                        pos += 1
                        idx += 1
                        n_split += 1
                idx += 1
    return n_split


def build_nc(bpc):
    nc = bass.Bass("TRN2", target_bir_lowering=False, debug=False,
                   num_devices=N_CORES)
    cf = nc.dram_tensor("cf", [bpc, C, T], F32, kind="ExternalInput").ap()
    qf = nc.dram_tensor("qf", [bpc, C, T], F32, kind="ExternalInput").ap()
    # stacked MLP weights: index 0 block = query branch, 1 block = class
    w1s = nc.dram_tensor("w1s", [T, 64], F32, kind="ExternalInput").ap()
    w2s = nc.dram_tensor("w2s", [64, T], F32, kind="ExternalInput").ap()
    b1s = nc.dram_tensor("b1s", [64, 1], F32, kind="ExternalInput").ap()
    b2s = nc.dram_tensor("b2s", [T, 2], F32, kind="ExternalInput").ap()
    onec = nc.dram_tensor("onec", [128, 1], F32, kind="ExternalInput").ap()
    imc = nc.dram_tensor("imc", [128, 1], F32, kind="ExternalInput").ap()
    oner = nc.dram_tensor("oner", [1, 128], F32, kind="ExternalInput").ap()
    i128 = nc.dram_tensor("i128", [128, 128], F32, kind="ExternalInput").ap()
    co = nc.dram_tensor("co", [bpc, C, T], F32, kind="ExternalOutput").ap()
    qo = nc.dram_tensor("qo", [bpc, C, T], F32, kind="ExternalOutput").ap()

    with tile.TileContext(nc) as tc:
        with (
            tc.tile_pool(name="const", bufs=1) as cp,
            tc.tile_pool(name="xp", bufs=3) as xp,
            tc.tile_pool(name="sqp", bufs=2) as sqp,
            tc.tile_pool(name="op", bufs=3) as op,
            tc.tile_pool(name="pqp", bufs=2) as pqp,
            tc.tile_pool(name="smp", bufs=3) as smp,
            tc.tile_pool(name="gps", bufs=1, space="PSUM") as gps,
            tc.tile_pool(name="rowps", bufs=2, space="PSUM") as rowps,
            tc.tile_pool(name="smallps", bufs=2, space="PSUM") as smallps,
            tc.tile_pool(name="bcps", bufs=1, space="PSUM") as bcps,
        ):
            # ---- persistent constants ----
            w1a = cp.tile([TA, 64], F32)
            w1b = cp.tile([TB, 64], F32)
            w2_sb = cp.tile([64, T], F32)
            b1_sb = cp.tile([64, 1], F32)
            b2a = cp.tile([TA, 2], F32)
            b2b = cp.tile([TB, 2], F32)
            onec_sb = cp.tile([128, 1], F32)
            imc_sb = cp.tile([128, 1], F32)
            oner_sb = cp.tile([1, 128], F32)
            id_sb = cp.tile([128, 128], F32)
            nc.sync.dma_start(w1a[:, :], w1s[0:TA, :])
            nc.sync.dma_start(w1b[:, :], w1s[TA:T, :])
            nc.sync.dma_start(w2_sb[:, :], w2s[:, :])
            nc.sync.dma_start(b1_sb[:, :], b1s[:, :])
            nc.sync.dma_start(b2a[:, :], b2s[0:TA, :])
            nc.sync.dma_start(b2b[:, :], b2s[TA:T, :])
            nc.sync.dma_start(onec_sb[:, :], onec[:, :])
            nc.sync.dma_start(imc_sb[:, :], imc[:, :])
            nc.sync.dma_start(oner_sb[:, :], oner[:, :])
            nc.sync.dma_start(id_sb[:, :], i128[:, :])

            for b in range(bpc):
                # ---- load feats: x[:, n, 0:196]=cls, [:, n, 196:392]=qry ----
                x = xp.tile([128, NCH, 2 * T], F32)
                nc.sync.dma_start(
                    x[:, :, 0:T], cf[b].rearrange("(n p) t -> p n t", p=128))
                nc.sync.dma_start(
                    x[:, :, T:2 * T], qf[b].rearrange("(n p) t -> p n t", p=128))

                # ---- squared feats (GPSIMD; SBUF only) ----
                sq = sqp.tile([128, NCH, 2 * T], F32)
                nc.gpsimd.tensor_mul(sq[:, :, :], x[:, :, :], x[:, :, :])

                # ---- ssq[t] = sum_c sq[c, t] for cls|qry -> [1, 392] ----
                ssq_ps = rowps.tile([1, 2 * T], F32, space="PSUM", tag="rowps")
                for n in range(NCH):
                    nc.tensor.matmul(ssq_ps[:, :], onec_sb[:, :], sq[:, n, :],
                                     start=(n == 0), stop=(n == NCH - 1))

                # ---- norms -> per-partition columns rc|rq ----
                nrm_row = smp.tile([1, 2 * T], F32)
                nc.scalar.sqrt(nrm_row[:, :], ssq_ps[:, :])
                nrmcol_ps = smallps.tile([128, 4], F32, space="PSUM", tag="smallps")
                one1 = onec_sb[0:1, 0:1]
                # cols: 0 = |cls| chunk a, 1 = |qry| a, 2 = |cls| b, 3 = |qry| b
                nc.tensor.matmul(nrmcol_ps[:, 0:1], nrm_row[:, 0:TA], one1,
                                 start=True, stop=True)
                nc.tensor.matmul(nrmcol_ps[:, 1:2], nrm_row[:, T:T + TA], one1,
                                 start=True, stop=True)
                nc.tensor.matmul(nrmcol_ps[0:TB, 2:3], nrm_row[:, TA:T], one1,
                                 start=True, stop=True)
                nc.tensor.matmul(nrmcol_ps[0:TB, 3:4], nrm_row[:, T + TA:2 * T],
                                 one1, start=True, stop=True)
                rcq_a = smp.tile([TA, 2], F32)
                rcq_b = smp.tile([TB, 2], F32)
                nc.vector.reciprocal(rcq_a[:, :], nrmcol_ps[:, 0:2])
                nc.vector.reciprocal(rcq_b[:, :], nrmcol_ps[0:TB, 2:4])

                # ---- raw grams G = cls^T @ qry (t x u), Gt = qry^T @ cls ----
                g_ps = gps.tile([128, 2 * T], F32, space="PSUM", tag="g")
                gt_ps = gps.tile([128, 2 * T], F32, space="PSUM", tag="gt")
                g_regions = [
                    (g_ps[0:TA, 0:T], slice(0, TA), slice(T, 2 * T)),
                    (g_ps[0:TB, T:2 * T], slice(TA, T), slice(T, 2 * T)),
                    (gt_ps[0:TA, 0:T], slice(T, T + TA), slice(0, T)),
                    (gt_ps[0:TB, T:2 * T], slice(T + TA, 2 * T), slice(0, T)),
                ]
                for out_ap, lsl_g, rsl_g in g_regions:
                    for n in range(NCH):
                        nc.tensor.matmul(out_ap, x[:, n, lsl_g], x[:, n, rsl_g],
                                         start=(n == 0), stop=(n == NCH - 1))

                # ---- P = diag(rc) G, Q = diag(rq) Gt  (ACT scaled copies) ----
                p_a = pqp.tile([TA, T], F32)
                p_b = pqp.tile([TB, T], F32)
                q_a = pqp.tile([TA, T], F32)
                q_b = pqp.tile([TB, T], F32)
                nc.scalar.mul(p_a[:, :], g_ps[0:TA, 0:T], rcq_a[:, 0:1])
                nc.scalar.mul(p_b[:, :], g_ps[0:TB, T:2 * T], rcq_b[:, 0:1])
                nc.scalar.mul(q_a[:, :], gt_ps[0:TA, 0:T], rcq_a[:, 1:2])
                nc.scalar.mul(q_b[:, :], gt_ps[0:TB, T:2 * T], rcq_b[:, 1:2])

                # ---- means: mcol_raw cols 0=mq_a 1=mc_a 2=mq_b 3=mc_b ----
                mcol_ps = smallps.tile([128, 4], F32, space="PSUM", tag="smallps")
                nc.tensor.matmul(mcol_ps[0:TA, 0:1], q_a[:, 0:TA],
                                 imc_sb[0:TA, :], start=True, stop=False)
                nc.tensor.matmul(mcol_ps[0:TA, 0:1], q_b[:, 0:TA],
                                 imc_sb[0:TB, :], start=False, stop=True)
                nc.tensor.matmul(mcol_ps[0:TA, 1:2], p_a[:, 0:TA],
                                 imc_sb[0:TA, :], start=True, stop=False)
                nc.tensor.matmul(mcol_ps[0:TA, 1:2], p_b[:, 0:TA],
                                 imc_sb[0:TB, :], start=False, stop=True)
                nc.tensor.matmul(mcol_ps[0:TB, 2:3], q_a[:, TA:T],
                                 imc_sb[0:TA, :], start=True, stop=False)
                nc.tensor.matmul(mcol_ps[0:TB, 2:3], q_b[:, TA:T],
                                 imc_sb[0:TB, :], start=False, stop=True)
                nc.tensor.matmul(mcol_ps[0:TB, 3:4], p_a[:, TA:T],
                                 imc_sb[0:TA, :], start=True, stop=False)
                nc.tensor.matmul(mcol_ps[0:TB, 3:4], p_b[:, TA:T],
                                 imc_sb[0:TB, :], start=False, stop=True)
                # finish means: mq *= rc, mc *= rq  (rcq cols are [rc, rq])
                mcol_a = smp.tile([TA, 2], F32)
                mcol_b = smp.tile([TB, 2], F32)
                nc.vector.tensor_mul(mcol_a[:, :], mcol_ps[0:TA, 0:2], rcq_a[:, :])
                nc.vector.tensor_mul(mcol_b[:, :], mcol_ps[0:TB, 2:4], rcq_b[:, :])

                # ---- MLP layer 1: h = W1s^T @ mcol  [64, 2] ----
                h_ps = smallps.tile([64, 2], F32, space="PSUM", tag="smallps")
                nc.tensor.matmul(h_ps[:, :], w1a[:, :], mcol_a[:, :],
                                 start=True, stop=False)
                nc.tensor.matmul(h_ps[:, :], w1b[:, :], mcol_b[:, :],
                                 start=False, stop=True)
                # relu into zeroed z: z[0:32,0]=q-branch, z[32:64,1]=c-branch
                z = smp.tile([64, 2], F32)
                nc.gpsimd.memset(z[:, :], 0.0)
                nc.scalar.activation(z[0:32, 0:1], h_ps[0:32, 0:1], AF.Relu,
                                     bias=b1_sb[0:32, :], scale=1.0)
                nc.scalar.activation(z[32:64, 1:2], h_ps[32:64, 1:2], AF.Relu,
                                     bias=b1_sb[32:64, :], scale=1.0)

                # ---- MLP layer 2: k = W2s^T @ z -> [196, 2] in 2 chunks ----
                k_ps = smallps.tile([128, 4], F32, space="PSUM", tag="smallps")
                nc.tensor.matmul(k_ps[0:TA, 0:2], w2_sb[:, 0:TA], z[:, :],
                                 start=True, stop=True)
                nc.tensor.matmul(k_ps[0:TB, 2:4], w2_sb[:, TA:T], z[:, :],
                                 start=True, stop=True)
                va = smp.tile([TA, 2], F32)
                vb = smp.tile([TB, 2], F32)
                nc.vector.tensor_add(va[:, :], k_ps[0:TA, 0:2], b2a[:, :])
                nc.vector.tensor_add(vb[:, :], k_ps[0:TB, 2:4], b2b[:, :])

                # ---- logits rows: [1, 0:196]=q-branch, [1, 196:392]=c ----
                lraw_ps = rowps.tile([1, 2 * T], F32, space="PSUM", tag="rowps")
                nc.tensor.matmul(lraw_ps[:, 0:T], va[:, 0:1], p_a[:, :],
                                 start=True, stop=False)
                nc.tensor.matmul(lraw_ps[:, 0:T], vb[:, 0:1], p_b[:, :],
                                 start=False, stop=True)
                nc.tensor.matmul(lraw_ps[:, T:2 * T], va[:, 1:2], q_a[:, :],
                                 start=True, stop=False)
                nc.tensor.matmul(lraw_ps[:, T:2 * T], vb[:, 1:2], q_b[:, :],
                                 start=False, stop=True)

                # ---- rc/rq back to rows: rrow = [rc_row | rq_row] ----
                rrow_ps = rowps.tile([1, 2 * T], F32, space="PSUM", tag="rowps")
                nc.tensor.matmul(rrow_ps[:, 0:TA], rcq_a[:, 0:1],
                                 id_sb[:, :], start=True, stop=True)
                nc.tensor.matmul(rrow_ps[:, TA:T], rcq_b[:, 0:1],
                                 id_sb[0:TB, 0:TB], start=True, stop=True)
                nc.tensor.matmul(rrow_ps[:, T:T + TA], rcq_a[:, 1:2],
                                 id_sb[:, :], start=True, stop=True)
                nc.tensor.matmul(rrow_ps[:, T + TA:2 * T], rcq_b[:, 1:2],
                                 id_sb[0:TB, 0:TB], start=True, stop=True)
                rr_sb = smp.tile([1, 2 * T], F32)
                nc.scalar.copy(rr_sb[:, :], rrow_ps[:, :])

                # ---- softmax per branch (row layout, partition 0) ----
                at1 = smp.tile([1, 2 * T], F32)  # [1+attn_c | 1+attn_q]
                for br in range(2):  # 0 = q-branch, 1 = c-branch
                    lsl = slice(br * T, br * T + T)          # lraw cols
                    rsl = slice((1 - br) * T, (1 - br) * T + T)  # rq for q-br
                    lg = smp.tile([1, T], F32, tag="lg", bufs=3)
                    mx = smp.tile([1, 1], F32, tag="mx", bufs=3)
                    nc.vector.tensor_mul(lg[:, :], lraw_ps[:, lsl], rr_sb[:, rsl])
                    nc.vector.reduce_max(mx[:, :], lg[:, :],
                                         axis=mybir.AxisListType.X)
                    nb = smp.tile([1, 1], F32, tag="nb", bufs=3)
                    nc.vector.tensor_scalar_mul(nb[:, :], mx[:, :], -INV_TEMP)
                    e = smp.tile([1, T], F32, tag="e", bufs=3)
                    sm = smp.tile([1, 1], F32, tag="sm", bufs=3)
                    nc.scalar.activation(e[:, :], lg[:, :], AF.Exp,
                                         bias=nb[:, :], scale=INV_TEMP,
                                         accum_out=sm[:, :])
                    rs = smp.tile([1, 1], F32, tag="rs", bufs=3)
                    nc.vector.reciprocal(rs[:, :], sm[:, :])
                    # 1 + attn; q-branch scales qry cols (196:392), c cls cols
                    osl = slice((1 - br) * T, (1 - br) * T + T)
                    nc.scalar.activation(at1[:, osl], e[:, :], AF.Identity,
                                         bias=1.0, scale=rs[:, :])

                # ---- broadcast rows across partitions, apply, store ----
                bc_ps = bcps.tile([128, 2 * T], F32, space="PSUM", tag="bc")
                nc.tensor.matmul(bc_ps[:, 0:T], oner_sb[:, :], at1[:, 0:T],
                                 start=True, stop=True)
                nc.tensor.matmul(bc_ps[:, T:2 * T], oner_sb[:, :], at1[:, T:2 * T],
                                 start=True, stop=True)
                o = op.tile([128, NCH, 2 * T], F32)
                for n in range(NCH):
                    nc.vector.tensor_mul(o[:, n, :], x[:, n, :], bc_ps[:, :])
                nc.sync.dma_start(
                    co[b].rearrange("(n p) t -> p n t", p=128), o[:, :, 0:T])
                nc.sync.dma_start(
                    qo[b].rearrange("(n p) t -> p n t", p=128), o[:, :, T:2 * T])
    _split_waits(nc)
    return nc


def _consts():
    return {
        "onec": np.ones((128, 1), np.float32),
        "imc": np.full((128, 1), 1.0 / T, np.float32),
        "oner": np.ones((1, 128), np.float32),
        "i128": np.eye(128, dtype=np.float32),
    }


_CACHE = {}


def prep_in_maps(class_feat, query_feat, cw1, cb1, cw2, cb2, qw1, qb1, qw2, qb2):
    B = class_feat.shape[0]
    bpc = B // N_CORES
    cfull = np.ascontiguousarray(np.asarray(class_feat, np.float32).reshape(B, C, T))
    qfull = np.ascontiguousarray(np.asarray(query_feat, np.float32).reshape(B, C, T))
    w1s = np.concatenate([np.asarray(qw1), np.asarray(cw1)], axis=1).astype(np.float32)
    w2s = np.concatenate([np.asarray(qw2), np.asarray(cw2)], axis=0).astype(np.float32)
    b1s = np.concatenate([np.asarray(qb1), np.asarray(cb1)])[:, None].astype(np.float32)
    b2s = np.stack([np.asarray(qb2), np.asarray(cb2)], axis=1).astype(np.float32)
    consts = _consts()
    in_maps = []
    for c in range(N_CORES):
        sl = slice(c * bpc, (c + 1) * bpc)
        in_maps.append({
            "cf": cfull[sl], "qf": qfull[sl],
            "w1s": w1s, "w2s": w2s, "b1s": b1s, "b2s": b2s, **consts,
        })
    return in_maps


def kernel(class_feat, query_feat, cw1, cb1, cw2, cb2, qw1, qb1, qw2, qb2):
    B = class_feat.shape[0]
    bpc = B // N_CORES
    if bpc not in _CACHE:
        _CACHE[bpc] = build_nc(bpc)
    nc = _CACHE[bpc]
    in_maps = prep_in_maps(class_feat, query_feat, cw1, cb1, cw2, cb2,
                           qw1, qb1, qw2, qb2)
    res = run_bass_kernel_spmd(nc, in_maps, core_ids=list(range(N_CORES)))
    S = int(np.sqrt(T))
    co = np.concatenate([res.results[c]["co"] for c in range(N_CORES)], axis=0)
    qo = np.concatenate([res.results[c]["qo"] for c in range(N_CORES)], axis=0)
    return (co.reshape(B, C, S, S), qo.reshape(B, C, S, S))
